# revision 1
# baseline (speedup 1.0000x reference)
"""TRN2 Bass kernel for nn_NMS (offset min-sum LDPC decoder, batch 256).

Self-contained: derives all index tables from the H input at call time,
shards the batch across 8 NeuronCores (32 per core), runs one SPMD Bass
program via run_bass_kernel_spmd, and gathers the full [256, 576] output.

Per-core layout: 128 partitions = 4 row-blocks x 32 batch; each row-block's
edges live on the free axis as [36 rows x 16 slots] (15 real + 1 pad).
Per decoding iteration:
  X = gather(Zrep) - E              (GPSIMD indirect_copy + DVE; iteration 0
                                     uses a host-precomputed X0 DMA)
  row min1/min2/sign-parity         (DVE segmented reduces, |.| fused into the
                                     min reduce; second-min via predicated BIG
                                     overwrite; ACT abs/sign)
  E = sign * select(mask, u2, u1)   (DVE predicated select)
  column sums via 2 strip gathers   (GPSIMD) + quarter adds (DVE)
  cross-block sum + 4x replicate    (PE one-hot fp32 matmul - exact for 0/1
                                     weights; rare deep-column edges via tiny
                                     accumulate-matmuls)
  Z = colsum + r                    (DVE)
Output = Z rows of partition block 0.

Multi-wait instructions are post-processed into standalone EventSemaphore
waits (hoist_waits) because this walrus build accepts only one sync-wait
slot per TPB instruction.
"""
import numpy as np
from contextlib import ExitStack

import concourse.bass as bass
import concourse.tile as tile
from concourse import mybir, library_config

FP32 = mybir.dt.float32
I32 = mybir.dt.int32
U16 = mybir.dt.uint16

P = 128
B = 32           # batch per core
NBLK = 4
RPB = 36         # rows per block
KPAD = 16        # padded row degree
ROW_DEG = 15
EPB = RPB * KPAD  # 576 edge slots per block
N = 576          # columns
D_STRIP = 4
D_KEEP = 3      # strips gathered; deeper edges via tiny accum-matmuls
ITERS = 3
BIG = np.float32(1e30)


# ---------------------------------------------------------------- tables ----
def build_tables(H):
    MROWS = H.shape[0]
    cols = np.array([np.nonzero(H[m])[0] for m in range(MROWS)], dtype=np.int64)
    assert cols.shape == (MROWS, ROW_DEG)
    coldeg = H.sum(0)
    heat = np.array([-coldeg[cols[m]].max() for m in range(MROWS)])
    order = list(np.argsort(heat, kind="stable"))
    blocks = [[] for _ in range(NBLK)]
    cnt = np.zeros((NBLK, N), dtype=np.int32)
    for m in order:
        best, bestkey = None, None
        for j in range(NBLK):
            if len(blocks[j]) >= RPB:
                continue
            key = tuple(np.sort(cnt[j, cols[m]])[::-1])
            if best is None or key < bestkey:
                best, bestkey = j, key
        blocks[best].append(m)
        cnt[best, cols[m]] += 1
    assign = np.zeros(MROWS, dtype=np.int64)
    for j, b in enumerate(blocks):
        for m in b:
            assign[m] = j

    def cost_vec(c):
        h = np.bincount(c.flatten(), minlength=8)
        return (int(h[5:].sum()), int(h[4]), int(h[3]))

    for _ in range(12):
        improved = False
        for m1 in range(MROWS):
            for m2 in range(m1 + 1, MROWS):
                j1, j2 = assign[m1], assign[m2]
                if j1 == j2:
                    continue
                c = cnt.copy()
                c[j1, cols[m1]] -= 1
                c[j1, cols[m2]] += 1
                c[j2, cols[m2]] -= 1
                c[j2, cols[m1]] += 1
                if cost_vec(c) < cost_vec(cnt):
                    assign[m1], assign[m2] = j2, j1
                    cnt = c
                    improved = True
        if not improved or cnt.max() <= D_STRIP:
            break
    assert cnt.max() <= D_STRIP, f"strip depth {cnt.max()} > {D_STRIP}"
    rows_of_block = [np.array([m for m in range(MROWS) if assign[m] == j],
                              dtype=np.int64) for j in range(NBLK)]

    colidx = np.full((NBLK, RPB, KPAD), N, dtype=np.int64)
    for j in range(NBLK):
        for mm, m in enumerate(rows_of_block[j]):
            colidx[j, mm, :ROW_DEG] = cols[m]

    strip_pos = np.full((NBLK, N, D_STRIP), EPB, dtype=np.int64)
    fill = np.zeros((NBLK, N), dtype=np.int64)
    for j in range(NBLK):
        for mm in range(RPB):
            for k in range(ROW_DEG):
                n = colidx[j, mm, k]
                d = fill[j, n]
                fill[j, n] = d + 1
                strip_pos[j, n, d] = mm * KPAD + k
    # overflow edges beyond D_KEEP strips -> handled by tiny accum-matmuls
    overflow = [(j, n, int(strip_pos[j, n, d]))
                for j in range(NBLK) for n in range(N)
                for d in range(D_KEEP, D_STRIP) if strip_pos[j, n, d] != EPB]
    assert len(overflow) <= 8, f"too many overflow edges: {len(overflow)}"

    # ---- wrapped int16 index tensors (per 16-partition core group) ----
    def wrap(vals_per_block, num_idxs):
        t = np.zeros((P, num_idxs // 16), dtype=np.uint16)
        for c in range(8):
            j = c // 2
            v = vals_per_block[j]
            for i in range(num_idxs):
                t[16 * c + i % 16, i // 16] = v[i]
        return t

    zvals = [colidx[j].reshape(-1) for j in range(NBLK)]           # 576 each
    # col-half-major: [h, d, c] so each half's strips are one contiguous
    # 864-element gather consumed right after
    svals = []
    for j in range(NBLK):
        v = np.empty(N * D_KEEP, dtype=np.int64)
        i = 0
        for h in range(2):
            for d in range(D_KEEP):
                for c in range(N // 2):
                    v[i] = strip_pos[j, h * (N // 2) + c, d]
                    i += 1
        svals.append(v)
    zidx = wrap(zvals, EPB)
    sidx = wrap(svals, N * D_KEEP)

    # one-hot cross-block sum + replicate: W[(j',b'), (j,b)] = (b'==b)
    wmat = np.zeros((P, P), dtype=np.float32)
    for jp in range(NBLK):
        for bp in range(B):
            for j in range(NBLK):
                wmat[jp * B + bp, j * B + bp] = 1.0
    return dict(zidx=zidx, sidx=sidx, wmat=wmat, colidx=colidx,
                strip_pos=strip_pos, overflow=overflow, Dmax=int(cnt.max()))


def build_x0(r_slice, colidx):
    """Host-side iteration-0 gather: x0[(j,b), (mm,k)] = r[b, col] (pads BIG)."""
    rpad = np.concatenate([r_slice, np.full((B, 1), BIG, np.float32)], axis=1)
    x0 = rpad[:, colidx]                      # [B, NBLK, RPB, KPAD]
    x0 = x0.transpose(1, 0, 2, 3).reshape(P, EPB)
    return np.ascontiguousarray(x0)


# ---------------------------------------------------------------- kernel ----
def hoist_waits(nc, max_embedded=1):
    """Split multi-wait instructions into standalone EventSemaphore waits.

    The walrus build used by the axon compile path only supports a single
    sync-wait slot on most TPB instruction structs; Tile attaches one wait
    per producer proc.  Hoist the extras onto the instruction's sequencer
    as separate wait instructions (exactly what raw-bass wait_ge emits).
    """
    k = 0
    for f in nc.m.functions:
        for b in f.blocks:
            insts = b.instructions
            out = []
            for i in insts:
                tname = type(i).__name__
                si = i.sync_info
                if (si is not None and tname != "InstEventSemaphore"
                        and len(si.on_wait) > max_embedded):
                    waits = list(si.on_wait)
                    keep = waits[:max_embedded]
                    for w in waits[max_embedded:]:
                        es = mybir.InstEventSemaphore(
                            name=f"hoistw{k}", ins=[], outs=[])
                        k += 1
                        es.engine = i.engine
                        es.sync_info = mybir.SyncInfo(on_wait=[w], on_update=[])
                        nc.inst_map[es.name] = es
                        out.append(es)
                    i.sync_info = mybir.SyncInfo(
                        on_wait=keep, on_update=list(si.on_update))
                out.append(i)
            b.instructions = out


def build_bass(alpha, beta, overflow=()):
    """alpha/beta: lists of 3 floats (baked as immediates).
    overflow: [(j, n, pos)] edges beyond D_KEEP strips, added via tiny
    accumulate-matmuls."""
    nc = bass.Bass("TRN2", target_bir_lowering=False, debug=False)
    r_d = nc.dram_tensor("r", [B, N], FP32, kind="ExternalInput")
    x0_d = nc.dram_tensor("x0", [P, EPB], FP32, kind="ExternalInput")
    zidx_d = nc.dram_tensor("zidx", [P, EPB // 16], U16, kind="ExternalInput")
    sidx_d = nc.dram_tensor("sidx", [P, N * D_KEEP // 16], U16, kind="ExternalInput")
    wmat_d = nc.dram_tensor("wmat", [P, P], FP32, kind="ExternalInput")
    out_d = nc.dram_tensor("out", [B, N], FP32, kind="ExternalOutput")
    HALF = N // 2

    with tile.TileContext(nc) as tc:
        with ExitStack() as ctx:
            pool = ctx.enter_context(tc.tile_pool(name="main", bufs=1))
            pspool = ctx.enter_context(tc.tile_pool(name="ps", bufs=1, space="PSUM"))

            r_rep = pool.tile([P, N + 4], FP32)  # col N = BIG pad
            zrep = pool.tile([P, N + 4], FP32)    # col N = BIG pad
            E = pool.tile([P, EPB + 4], FP32)     # col EPB = zero slot
            Xg = pool.tile([P, EPB], FP32)
            X = pool.tile([P, RPB, KPAD], FP32)
            A3 = pool.tile([P, RPB, KPAD + 1], FP32)   # strided
            sgn = pool.tile([P, RPB, KPAD], FP32)
            mask3 = pool.tile([P, RPB, KPAD + 1], mybir.dt.uint8)  # strided, int mask
            bigc = pool.tile([P, 1], FP32)
            Etmp3 = pool.tile([P, RPB, KPAD + 1], FP32)   # strided
            minp = pool.tile([P, 2, RPB], FP32)
            up = pool.tile([P, 2, RPB], FP32)
            ssum = pool.tile([P, RPB], FP32)
            cntf = pool.tile([P, RPB], FP32)
            cnti = pool.tile([P, RPB], I32)
            sprod = pool.tile([P, RPB], FP32)
            G = pool.tile([P, 2, D_KEEP, N // 2], FP32)   # [half, strip, col]
            csum = pool.tile([P, N], FP32)
            zidx = pool.tile([P, EPB // 16], U16)
            sidx = pool.tile([P, N * D_KEEP // 16], U16)
            wmat = pool.tile([P, P], FP32)
            zpsA = pspool.tile([P, HALF], FP32)
            zpsB = pspool.tile([P, HALF], FP32)

            # ---- static loads ----
            # r replicated 4x into zrep / r_rep via single broadcast DMAs
            r_bc = bass.AP(tensor=r_d.ap().tensor, offset=0,
                           ap=[[0, NBLK], [N, B], [1, N]])
            nc.scalar.dma_start(wmat[:], wmat_d[:])
            nc.gpsimd.dma_start(zidx[:], zidx_d[:])
            nc.gpsimd.dma_start(sidx[:], sidx_d[:])
            nc.vector.memset(zrep[:, N:N + 1], float(BIG))
            nc.vector.memset(bigc[:], float(BIG))
            nc.vector.memset(E[:, EPB:EPB + 1], 0.0)
            # consume index-table DMA deps on Pool; keeps the first gather to
            # a single extra wait and warms the wmat path
            idxtouch = pool.tile([P, 2], U16)
            nc.gpsimd.tensor_copy(idxtouch[:, 0:1], zidx[:, 0:1])
            nc.tensor.matmul(zpsA[0:1, 0:1], lhsT=wmat[0:B, 0:1],
                             rhs=wmat[0:B, 0:1], start=True, stop=True)

            Xf = X[:].rearrange("p a b -> p (a b)")
            Gf = G[:].rearrange("p a b c -> p (a b c)")
            Ev = E[:, 0:EPB].rearrange("p (a b) -> p a b", a=RPB)
            RH = RPB // 2          # rows per half (per block)
            EH = RH * KPAD         # edge slots per half = 288
            ovf_by_half = ([], [])
            for (j0, n0, p0) in overflow:
                ovf_by_half[n0 // HALF].append((j0, n0 % HALF, p0))

            def rsl(h):            # row slice of a half
                return slice(RH * h, RH * (h + 1))

            for it in range(ITERS):
                al = float(alpha[it])
                ab = float(alpha[it] * beta[it])

                # ---- X = gather(Zrep) - E  (it 0: host-precomputed X0) ----
                if it == 0:
                    nc.sync.dma_start(Xf, x0_d[:])
                    nc.sync.dma_start(r_rep[:, 0:N], r_bc)
                else:
                    nc.gpsimd.indirect_copy(Xg[:], zrep[:, 0:N + 1], zidx[:], True)
                    nc.vector.tensor_sub(Xf, Xg[:], E[:, 0:EPB])
                nc.scalar.activation(A3[:, :, 0:KPAD], X[:],
                                     func=mybir.ActivationFunctionType.Abs)
                nc.scalar.activation(sgn[:], X[:],
                                     func=mybir.ActivationFunctionType.Sign)

                # ---- phase 1: row stats (full width) ----
                nc.vector.tensor_reduce(minp[:, 0, :], X[:],
                                        axis=mybir.AxisListType.X,
                                        op=mybir.AluOpType.min,
                                        apply_absolute_value=True)
                min1b = minp[:, 0, :].unsqueeze(2).broadcast_to([P, RPB, KPAD])
                nc.vector.tensor_tensor(mask3[:, :, 0:KPAD], A3[:, :, 0:KPAD],
                                        min1b, op=mybir.AluOpType.is_equal)
                bigb = bigc[:].unsqueeze(2).broadcast_to([P, RPB, KPAD])
                nc.vector.copy_predicated(A3[:, :, 0:KPAD], mask3[:, :, 0:KPAD],
                                          bigb)
                nc.vector.tensor_reduce(minp[:, 1, :], A3[:, :, 0:KPAD],
                                        axis=mybir.AxisListType.X,
                                        op=mybir.AluOpType.min)
                # sign parity (off critical path; interleaves on DVE)
                nc.vector.tensor_reduce(ssum[:], sgn[:],
                                        axis=mybir.AxisListType.X,
                                        op=mybir.AluOpType.add)
                nc.vector.tensor_scalar(cntf[:], ssum[:], -0.5, 8.0,
                                        op0=mybir.AluOpType.mult,
                                        op1=mybir.AluOpType.add)
                nc.vector.tensor_copy(cnti[:], cntf[:])
                nc.vector.tensor_single_scalar(cnti[:], cnti[:], 1,
                                               op=mybir.AluOpType.bitwise_and)
                nc.vector.tensor_scalar(sprod[:], cnti[:], -2.0, 1.0,
                                        op0=mybir.AluOpType.mult,
                                        op1=mybir.AluOpType.add)
                # u = relu(alpha*min - alpha*beta) * sprod, fused
                nc.vector.tensor_scalar(up[:], minp[:], al, ab,
                                        op0=mybir.AluOpType.mult,
                                        op1=mybir.AluOpType.subtract)
                sprodb2 = sprod[:].unsqueeze(1).broadcast_to([P, 2, RPB])
                nc.vector.scalar_tensor_tensor(up[:], up[:], 0.0, sprodb2,
                                               op0=mybir.AluOpType.max,
                                               op1=mybir.AluOpType.mult)

                # ---- phase 2: E = sgn * select(mask, u2, u1) ----
                u1b = up[:, 0, :].unsqueeze(2).broadcast_to([P, RPB, KPAD])
                u2b = up[:, 1, :].unsqueeze(2).broadcast_to([P, RPB, KPAD])
                nc.vector.tensor_copy(Etmp3[:, :, 0:KPAD], u1b)
                nc.vector.copy_predicated(Etmp3[:, :, 0:KPAD],
                                          mask3[:, :, 0:KPAD], u2b)
                nc.vector.tensor_mul(Ev[:], Etmp3[:, :, 0:KPAD], sgn[:])

                # ---- column sums: per col-half gather + adds + mm + z ----
                for h in range(2):
                    o = 864 * h
                    nc.gpsimd.indirect_copy(
                        Gf[:, o:o + 864], E[:, 0:EPB + 1],
                        sidx[:, o // 16:o // 16 + 54], True)
                for h in range(2):
                    sl = slice(h * HALF, (h + 1) * HALF)
                    zps = zpsA if h == 0 else zpsB
                    ovf = ovf_by_half[h]
                    for q in range(2):
                        qs = slice(h * HALF + q * 144, h * HALF + q * 144 + 144)
                        gq = slice(q * 144, q * 144 + 144)
                        nc.vector.tensor_add(csum[:, qs], G[:, h, 0, gq],
                                             G[:, h, 1, gq])
                        nc.vector.tensor_add(csum[:, qs], csum[:, qs],
                                             G[:, h, 2, gq])
                        nc.tensor.matmul(zps[:, q * 144:q * 144 + 144],
                                         lhsT=wmat[:], rhs=csum[:, qs],
                                         start=(q == 0),
                                         stop=(q == 1 and len(ovf) == 0))
                    for i, (j0, nn, p0) in enumerate(ovf):
                        nc.tensor.matmul(zps[:, nn:nn + 1],
                                         lhsT=wmat[32 * j0:32 * (j0 + 1), :],
                                         rhs=E[32 * j0:32 * (j0 + 1), p0:p0 + 1],
                                         start=False, stop=(i == len(ovf) - 1),
                                         tile_position=(32 * j0, 0))
                    nc.vector.tensor_add(zrep[:, sl], zps[:], r_rep[:, sl])
                    if it == ITERS - 1:
                        nc.sync.dma_start(out_d[:, sl], zrep[0:B, sl])

    hoist_waits(nc)
    return nc


# ------------------------------------------------------------ host driver ----
def shard_inputs(r_full, tables):
    """Returns per-core in_maps list."""
    in_maps = []
    for c in range(8):
        in_maps.append({
            "r": np.ascontiguousarray(r_full[c * B:(c + 1) * B]),
            "zidx": tables["zidx"],
            "sidx": tables["sidx"],
            "wmat": tables["wmat"],
        })
    return in_maps


_CACHE = {}


def kernel(r, H, alpha, beta):
    r = np.asarray(r, dtype=np.float32)
    H = np.asarray(H, dtype=np.float32)
    alpha_l = [float(x) for x in np.asarray(alpha).reshape(-1)]
    beta_l = [float(x) for x in np.asarray(beta).reshape(-1)]

    key = (H.tobytes(), tuple(alpha_l), tuple(beta_l))
    if key not in _CACHE:
        tables = build_tables(H)
        nc = build_bass(alpha_l, beta_l, tables["overflow"])
        _CACHE[key] = (tables, nc)
    tables, nc = _CACHE[key]

    from concourse.bass_utils import run_bass_kernel_spmd
    in_maps = []
    for c in range(8):
        rs = np.ascontiguousarray(r[c * B:(c + 1) * B])
        in_maps.append({
            "r": rs,
            "x0": build_x0(rs, tables["colidx"]),
            "zidx": tables["zidx"],
            "sidx": tables["sidx"],
            "wmat": tables["wmat"],
        })
    # the first execution on a freshly-attached device occasionally fails
    # with NRT_EXEC_UNIT_UNRECOVERABLE; a retry succeeds
    last = None
    for _attempt in range(3):
        try:
            res = run_bass_kernel_spmd(nc, in_maps, core_ids=list(range(8)))
            break
        except Exception as e:  # noqa: BLE001
            last = e
    else:
        raise last
    out = np.concatenate([res.results[c]["out"] for c in range(8)], axis=0)
    return out.astype(np.float32)

